# revision 18
# baseline (speedup 1.0000x reference)
"""Trainium2 Bass kernel for nn_Antecedents: fuzzy-rule antecedent activations.

Computes out[n, r] = prod_v memberships[v, n, set_v(r)] over the full
Cartesian product of fuzzy sets (R = 4**6 = 4096 rules), for N = 16384
samples, data-parallel over 8 NeuronCores (2048 samples per core).

The kernel is output-DMA-bound (16.78 MB bf16 per core, vs ~0.3 MB of
input), so the design minimizes input-stream bytes and keeps the
PE/ACT/DVE pipeline producing output tiles ahead of the DMA drain:

 * v0 never enters the log-space matmul: the one-hot matmul spans only
   v1..v5 (K = 40 = 5 vars x 4 sets x hi/lo bf16 split; the one-hot
   rhs is 1024 wide), and the v0 factor is applied by DVE
   tensor_scalar ops (the fast per-partition-scalar path) from raw f32
   memberships.  This cuts the input stream from ~1 MB (56-row hi/lo
   lcin + 4096-wide one-hot) to ~330 KB.

 * j0 ships via a pure-DVE product chain that needs only a 24-column
   f32 block: first output bytes ~2.5 us after the first input DMA
   lands, while the PE-path inputs are still in flight.

 * j1..j3 are PE-path singles (2 matmuls -> 1024-wide S in PSUM, one
   Exp, 4 DVE finals) with chunked output DMAs - they fill the stream
   during pipeline warmup.

 * j4..j15 are pairs: PE computes S for two js into one [128, 2048]
   PSUM tile, ACT drains it with a single Exp -> e2048 bf16, DVE
   broadcasts x X0[s0] into a [128, 8192] tile whose halves post as
   1 MB DMAs the moment each j's finals are done.

 * Input DMAs ride the scalar-queue HWDGE ring (except xa, first on
   the sync ring), so the sync ring's FIFO head frees up early for the
   first output dispatch; output DMAs alternate between the sync and
   scalar rings so the 16 SDMA engines round-robin two descriptor
   queues.  LC is split so j1's columns clear their completion receipt
   early and unblock the first matmul.

Sample layout: n = p*16 + j with p = SBUF/PSUM partition; lhsT for j
is the contiguous column block LC[0:40, (j-1)*128:j*128].  Output is
bf16 (rel err ~1.1e-2 vs the 2e-2 gate), upcast to f32 on the host.
"""

import numpy as np
from contextlib import ExitStack

import concourse.bass as bass
import concourse.tile as tile
from concourse import bacc, mybir
from concourse.bass_utils import run_bass_kernel_spmd

N_VARS = 6
N_FULL = 16384
N_SETS = 4
N_CORES = 8
N_SHARD = N_FULL // N_CORES  # 2048
P = 128
J = N_SHARD // P             # 16 samples per partition
R = N_SETS ** N_VARS         # 4096
F32 = mybir.dt.float32
BF16 = mybir.dt.bfloat16
MUL = mybir.AluOpType.mult
EXP = mybir.ActivationFunctionType.Exp

KK = 40           # lhsT rows: v1..v5 hi (20) | lo (20)
XIN_W = 128       # [j0 vars (24) | X0 j1..15 (60) | pad to 512B/part]
B_PAIRS = ((4, 5), (6, 7), (8, 9), (10, 11), (12, 13), (14, 15))

LAST_RESULTS = None
_CACHE = {}


def _ap(base, col, dims):
    """AP at column offset `col` of a [128, W] tile with free dims
    [(stride, count), ...] (stride 0 = broadcast)."""
    return bass.AP(
        tensor=base.tensor,
        offset=base.offset + col,
        ap=[base.ap[0]] + [[st, c] for (st, c) in dims],
    )


def build_nc():
    nc = bacc.Bacc(
        "TRN2", target_bir_lowering=False, debug=False, num_devices=N_CORES
    )
    xin = nc.dram_tensor("xin", [P, XIN_W], F32, kind="ExternalInput").ap()
    lcin = nc.dram_tensor("lcin", [KK, (J - 1) * P], BF16,
                          kind="ExternalInput").ap()
    ohin = nc.dram_tensor("ohin", [KK, 1024], BF16, kind="ExternalInput").ap()
    out = nc.dram_tensor("out", [N_SHARD, R], BF16, kind="ExternalOutput").ap()
    out_v = out.rearrange("(p f) r -> p (f r)", p=P)  # [128, J*R]

    with tile.TileContext(nc) as tc, ExitStack() as ctx:
        xpool = ctx.enter_context(tc.tile_pool(name="x", bufs=1))
        spool = ctx.enter_context(tc.tile_pool(name="scratch", bufs=2))
        epool = ctx.enter_context(tc.tile_pool(name="e", bufs=3))
        o1pool = ctx.enter_context(tc.tile_pool(name="o1", bufs=3))
        o2pool = ctx.enter_context(tc.tile_pool(name="o2", bufs=3))
        ppool = ctx.enter_context(tc.psum_pool(name="pp", bufs=2))

        # xa first on the sync ring (gates the DVE fast path); the
        # PE-path inputs on the scalar ring, LC split so j1..j3's
        # columns (and hence the first matmuls) clear their completion
        # receipt early.
        xa = xpool.tile([P, XIN_W], F32, tag="xa")
        nc.sync.dma_start(out=xa[:], in_=xin)
        ohB = xpool.tile([KK, 1024], BF16, tag="ohb")
        nc.scalar.dma_start(out=ohB[:, 0:512], in_=ohin[:, 0:512])
        LC = xpool.tile([KK, (J - 1) * P], BF16, tag="LC")
        nc.sync.dma_start(out=LC[:, 0:128], in_=lcin[:, 0:128])
        nc.scalar.dma_start(out=ohB[:, 512:], in_=ohin[:, 512:])
        nc.scalar.dma_start(out=LC[:, 128:384], in_=lcin[:, 128:384])
        nc.scalar.dma_start(out=LC[:, 384:], in_=lcin[:, 384:])
        xb = xa[:]

        def x0c(j, s):
            # X0 column: j0 keeps its full 24-col var block.
            col = s if j == 0 else 24 + (j - 1) * 4 + s
            return xa[:, col:col + 1]

        def lhsT(j):
            return LC[0:KK, (j - 1) * P:j * P]

        def emit_dve_j(j, n_chunks, q):
            # pure-DVE product chain for j0 (needs only its xa block).
            base = j * 24
            a16 = spool.tile([P, 16], F32, tag="a16")
            nc.vector.tensor_tensor(
                out=a16[:].rearrange("p (a b) -> p a b", a=4),
                in0=_ap(xb, base + 16, [(1, 4), (0, 4)]),
                in1=_ap(xb, base + 20, [(0, 4), (1, 4)]),
                op=MUL,
            )
            x23 = spool.tile([P, 16], F32, tag="x23")
            nc.vector.tensor_tensor(
                out=x23[:].rearrange("p (a b) -> p a b", a=4),
                in0=_ap(xb, base + 8, [(1, 4), (0, 4)]),
                in1=_ap(xb, base + 12, [(0, 4), (1, 4)]),
                op=MUL,
            )
            a256 = spool.tile([P, 256], BF16, tag="a256")
            nc.vector.tensor_tensor(
                out=a256[:].rearrange("p (g k) -> p g k", g=16),
                in0=_ap(x23[:], 0, [(1, 16), (0, 16)]),
                in1=_ap(a16[:], 0, [(0, 16), (1, 16)]),
                op=MUL,
            )
            a1024 = spool.tile([P, 1024], BF16, tag="a1024")
            for s1 in range(N_SETS):
                nc.vector.tensor_scalar_mul(
                    a1024[:, 256 * s1:256 * (s1 + 1)], a256[:],
                    xa[:, base + 4 + s1:base + 5 + s1],
                )
            ot = o1pool.tile([P, R], BF16, tag="o1")
            w = R // n_chunks
            for c in range(n_chunks):
                for s in range(c * N_SETS // n_chunks,
                               (c + 1) * N_SETS // n_chunks):
                    nc.vector.tensor_scalar_mul(
                        ot[:, 1024 * s:1024 * (s + 1)], a1024[:], x0c(j, s)
                    )
                q.dma_start(
                    out=out_v[:, j * R + c * w:j * R + (c + 1) * w],
                    in_=ot[:, c * w:(c + 1) * w],
                )

        def emit_single(j, n_chunks, q, split_first=False):
            # PE-path single j: 1024-wide S, Exp drain, 4 DVE finals.
            ps = ppool.tile([P, 2048], F32, tag="ps")
            for c in range(2):
                nc.tensor.matmul(
                    out=ps[:, c * 512:(c + 1) * 512],
                    lhsT=lhsT(j),
                    rhs=ohB[:, c * 512:(c + 1) * 512],
                    start=True,
                    stop=True,
                )
            e1024 = epool.tile([P, 1024], BF16, tag="e1024")
            ot = o1pool.tile([P, R], BF16, tag="o1")
            if split_first:
                # pipeline the first 1024-block at 512 granularity: each
                # exp half is multiplied and posted without waiting for
                # the other half (cuts ~0.6us off the first output byte
                # of the PE path during warmup).
                for h in range(2):
                    hw = slice(512 * h, 512 * (h + 1))
                    nc.scalar.activation(e1024[:, hw], ps[:, hw], EXP)
                    nc.vector.tensor_scalar_mul(
                        ot[:, hw], e1024[:, hw], x0c(j, 0))
                    q.dma_start(out=out_v[:, j * R + 512 * h:
                                          j * R + 512 * (h + 1)],
                                in_=ot[:, hw])
                for s in range(1, N_SETS):
                    nc.vector.tensor_scalar_mul(
                        ot[:, 1024 * s:1024 * (s + 1)], e1024[:], x0c(j, s))
                    q.dma_start(
                        out=out_v[:, j * R + 1024 * s:j * R + 1024 * (s + 1)],
                        in_=ot[:, 1024 * s:1024 * (s + 1)],
                    )
                return
            nc.scalar.activation(e1024[:], ps[:, 0:1024], EXP)
            w = R // n_chunks
            for c in range(n_chunks):
                for s in range(c * N_SETS // n_chunks,
                               (c + 1) * N_SETS // n_chunks):
                    nc.vector.tensor_scalar_mul(
                        ot[:, 1024 * s:1024 * (s + 1)], e1024[:], x0c(j, s)
                    )
                q.dma_start(
                    out=out_v[:, j * R + c * w:j * R + (c + 1) * w],
                    in_=ot[:, c * w:(c + 1) * w],
                )

        def emit_pair(ja, jb):
            ps = ppool.tile([P, 2048], F32, tag="ps")
            for idx, j in enumerate((ja, jb)):
                for c in range(2):
                    col = idx * 1024 + c * 512
                    nc.tensor.matmul(
                        out=ps[:, col:col + 512],
                        lhsT=lhsT(j),
                        rhs=ohB[:, c * 512:(c + 1) * 512],
                        start=True,
                        stop=True,
                    )
            e2048 = epool.tile([P, 2048], BF16, tag="e2048")
            nc.scalar.activation(e2048[:], ps[:], EXP)
            ot = o2pool.tile([P, 2 * R], BF16, tag="o2")
            for idx, j in enumerate((ja, jb)):
                for s in range(N_SETS):
                    nc.vector.tensor_scalar_mul(
                        ot[:, idx * R + 1024 * s:idx * R + 1024 * (s + 1)],
                        e2048[:, idx * 1024:(idx + 1) * 1024],
                        x0c(j, s),
                    )
                # post each j's 1 MB as soon as its finals are done, on
                # alternating rings: keeps the stream fed, keeps per-DMA
                # engine shares small, and lets the SDMA engines
                # round-robin between two descriptor queues.
                q = nc.sync if idx == 0 else nc.scalar
                q.dma_start(
                    out=out_v[:, j * R:(j + 1) * R],
                    in_=ot[:, idx * R:(idx + 1) * R],
                )

        emit_dve_j(0, n_chunks=4, q=nc.sync)
        emit_single(1, n_chunks=4, q=nc.scalar, split_first=True)
        emit_single(2, n_chunks=2, q=nc.sync)
        emit_single(3, n_chunks=2, q=nc.scalar)
        for pr in B_PAIRS:
            emit_pair(*pr)

    nc.compile()
    return nc


def _get_nc():
    if "nc" not in _CACHE:
        _CACHE["nc"] = build_nc()
    return _CACHE["nc"]


def _onehot() -> np.ndarray:
    """[40, 1024] bf16: rows v1..v5 hi (20) then lo (20); col r encodes
    (s1..s5) with s5 fastest."""
    import ml_dtypes

    r = np.arange(1024)
    o20 = np.zeros((20, 1024), dtype=np.float32)
    for v in range(1, N_VARS):
        sv = (r >> (2 * (N_VARS - 1 - v))) & 3
        for s in range(N_SETS):
            o20[(v - 1) * N_SETS + s] = (sv == s).astype(np.float32)
    return np.concatenate([o20, o20], axis=0).astype(ml_dtypes.bfloat16)


def _lcin(shard: np.ndarray) -> np.ndarray:
    """[40, 1920] bf16 log-domain hi/lo for v1..v5, j-major columns
    (col (j-1)*128+p = sample p*16+j); j0 (pure-DVE path) dropped."""
    import ml_dtypes

    t = shard[1:].transpose(0, 2, 1).reshape(20, N_SHARD)  # [(v,s), n]
    L = np.log(np.maximum(t, 1e-38)).astype(np.float32)
    hi = L.astype(ml_dtypes.bfloat16)
    lo = (L - hi.astype(np.float32)).astype(ml_dtypes.bfloat16)
    full = np.concatenate([hi, lo], axis=0)  # [40, n]
    # n = p*16 + j  ->  column (j-1)*128 + p
    full = full.reshape(KK, P, J).transpose(0, 2, 1).reshape(KK, N_SHARD)
    return np.ascontiguousarray(full[:, P:])


def _xin(shard: np.ndarray) -> np.ndarray:
    """[128, 128] f32: j0's 24-col var block | X0 for j1..15 | pad."""
    x = np.zeros((P, XIN_W), dtype=np.float32)
    m = shard.reshape(N_VARS, P, J, N_SETS)  # [v, p, j, s]
    x[:, 0:24] = m[:, :, 0, :].transpose(1, 0, 2).reshape(P, 24)
    x[:, 24:84] = m[0, :, 1:, :].reshape(P, 60)
    return np.ascontiguousarray(x)


def kernel(memberships):
    global LAST_RESULTS
    m = np.ascontiguousarray(np.asarray(memberships, dtype=np.float32))
    assert m.shape == (N_VARS, N_FULL, N_SETS), m.shape
    nc = _get_nc()
    oh = _onehot()
    shards = np.split(m, N_CORES, axis=1)
    in_maps = [
        {"xin": _xin(s), "lcin": _lcin(s), "ohin": oh} for s in shards
    ]
    res = run_bass_kernel_spmd(nc, in_maps, core_ids=list(range(N_CORES)))
    LAST_RESULTS = res
    return np.concatenate(
        [res.results[i]["out"] for i in range(N_CORES)], axis=0
    ).astype(np.float32)


# revision 19
# speedup vs baseline: 1.0288x; 1.0288x over previous
"""Trainium2 Bass kernel for nn_Antecedents: fuzzy-rule antecedent activations.

Computes out[n, r] = prod_v memberships[v, n, set_v(r)] over the full
Cartesian product of fuzzy sets (R = 4**6 = 4096 rules), for N = 16384
samples, data-parallel over 8 NeuronCores (2048 samples per core).

The kernel is output-DMA-bound (16.78 MB bf16 per core, vs ~0.3 MB of
input), so the design minimizes input-stream bytes and keeps the
PE/ACT/DVE pipeline producing output tiles ahead of the DMA drain:

 * v0 never enters the log-space matmul: the one-hot matmul spans only
   v1..v5 (K = 40 = 5 vars x 4 sets x hi/lo bf16 split; the one-hot
   rhs is 1024 wide), and the v0 factor is applied by DVE
   tensor_scalar ops (the fast per-partition-scalar path) from raw f32
   memberships.  This cuts the input stream from ~1 MB (56-row hi/lo
   lcin + 4096-wide one-hot) to ~330 KB.

 * j0 ships via a pure-DVE product chain that needs only a 24-column
   f32 block: first output bytes ~2.5 us after the first input DMA
   lands, while the PE-path inputs are still in flight.

 * j1..j3 are PE-path singles (2 matmuls -> 1024-wide S in PSUM, one
   Exp, 4 DVE finals) with chunked output DMAs - they fill the stream
   during pipeline warmup.

 * j4..j15 are pairs: PE computes S for two js into one [128, 2048]
   PSUM tile, ACT drains it with a single Exp -> e2048 bf16, DVE
   broadcasts x X0[s0] into a [128, 8192] tile whose halves post as
   1 MB DMAs the moment each j's finals are done.

 * Input DMAs ride the scalar-queue HWDGE ring (except xa, first on
   the sync ring), so the sync ring's FIFO head frees up early for the
   first output dispatch; output DMAs alternate between the sync and
   scalar rings so the 16 SDMA engines round-robin two descriptor
   queues.  LC is split so j1's columns clear their completion receipt
   early and unblock the first matmul.

Sample layout: n = p*16 + j with p = SBUF/PSUM partition; lhsT for j
is the contiguous column block LC[0:40, (j-1)*128:j*128].  Output is
bf16 (rel err ~1.1e-2 vs the 2e-2 gate), upcast to f32 on the host.
"""

import numpy as np
from contextlib import ExitStack

import concourse.bass as bass
import concourse.tile as tile
from concourse import bacc, mybir
from concourse.bass_utils import run_bass_kernel_spmd

N_VARS = 6
N_FULL = 16384
N_SETS = 4
N_CORES = 8
N_SHARD = N_FULL // N_CORES  # 2048
P = 128
J = N_SHARD // P             # 16 samples per partition
R = N_SETS ** N_VARS         # 4096
F32 = mybir.dt.float32
BF16 = mybir.dt.bfloat16
MUL = mybir.AluOpType.mult
EXP = mybir.ActivationFunctionType.Exp

KK = 40           # lhsT rows: v1..v5 hi (20) | lo (20)
XIN_W = 128       # [j0 vars (24) | X0 j1..15 (60) | pad to 512B/part]
B_PAIRS = ((4, 5), (6, 7), (8, 9), (10, 11), (12, 13), (14, 15))

LAST_RESULTS = None
_CACHE = {}


def _ap(base, col, dims):
    """AP at column offset `col` of a [128, W] tile with free dims
    [(stride, count), ...] (stride 0 = broadcast)."""
    return bass.AP(
        tensor=base.tensor,
        offset=base.offset + col,
        ap=[base.ap[0]] + [[st, c] for (st, c) in dims],
    )


def build_nc():
    nc = bacc.Bacc(
        "TRN2", target_bir_lowering=False, debug=False, num_devices=N_CORES
    )
    xin = nc.dram_tensor("xin", [P, XIN_W], F32, kind="ExternalInput").ap()
    lcin = nc.dram_tensor("lcin", [KK, (J - 1) * P], BF16,
                          kind="ExternalInput").ap()
    ohin = nc.dram_tensor("ohin", [KK, 1024], BF16, kind="ExternalInput").ap()
    out = nc.dram_tensor("out", [N_SHARD, R], BF16, kind="ExternalOutput").ap()
    out_v = out.rearrange("(p f) r -> p (f r)", p=P)  # [128, J*R]

    with tile.TileContext(nc) as tc, ExitStack() as ctx:
        xpool = ctx.enter_context(tc.tile_pool(name="x", bufs=1))
        spool = ctx.enter_context(tc.tile_pool(name="scratch", bufs=2))
        epool = ctx.enter_context(tc.tile_pool(name="e", bufs=3))
        o1pool = ctx.enter_context(tc.tile_pool(name="o1", bufs=3))
        o2pool = ctx.enter_context(tc.tile_pool(name="o2", bufs=3))
        ppool = ctx.enter_context(tc.psum_pool(name="pp", bufs=2))

        # xa first on the sync ring (gates the DVE fast path); the
        # PE-path inputs on the scalar ring, LC split so j1..j3's
        # columns (and hence the first matmuls) clear their completion
        # receipt early.
        xa = xpool.tile([P, XIN_W], F32, tag="xa")
        nc.sync.dma_start(out=xa[:], in_=xin)
        ohB = xpool.tile([KK, 1024], BF16, tag="ohb")
        nc.scalar.dma_start(out=ohB[:], in_=ohin)
        LC = xpool.tile([KK, (J - 1) * P], BF16, tag="LC")
        nc.scalar.dma_start(out=LC[:, 0:128], in_=lcin[:, 0:128])
        nc.scalar.dma_start(out=LC[:, 128:384], in_=lcin[:, 128:384])
        nc.scalar.dma_start(out=LC[:, 384:], in_=lcin[:, 384:])
        xb = xa[:]

        def x0c(j, s):
            # X0 column: j0 keeps its full 24-col var block.
            col = s if j == 0 else 24 + (j - 1) * 4 + s
            return xa[:, col:col + 1]

        def lhsT(j):
            return LC[0:KK, (j - 1) * P:j * P]

        def emit_dve_j(j, n_chunks, q):
            # pure-DVE product chain for j0 (needs only its xa block).
            base = j * 24
            a16 = spool.tile([P, 16], F32, tag="a16")
            nc.vector.tensor_tensor(
                out=a16[:].rearrange("p (a b) -> p a b", a=4),
                in0=_ap(xb, base + 16, [(1, 4), (0, 4)]),
                in1=_ap(xb, base + 20, [(0, 4), (1, 4)]),
                op=MUL,
            )
            x23 = spool.tile([P, 16], F32, tag="x23")
            nc.vector.tensor_tensor(
                out=x23[:].rearrange("p (a b) -> p a b", a=4),
                in0=_ap(xb, base + 8, [(1, 4), (0, 4)]),
                in1=_ap(xb, base + 12, [(0, 4), (1, 4)]),
                op=MUL,
            )
            a256 = spool.tile([P, 256], BF16, tag="a256")
            nc.vector.tensor_tensor(
                out=a256[:].rearrange("p (g k) -> p g k", g=16),
                in0=_ap(x23[:], 0, [(1, 16), (0, 16)]),
                in1=_ap(a16[:], 0, [(0, 16), (1, 16)]),
                op=MUL,
            )
            a1024 = spool.tile([P, 1024], BF16, tag="a1024")
            for s1 in range(N_SETS):
                nc.vector.tensor_scalar_mul(
                    a1024[:, 256 * s1:256 * (s1 + 1)], a256[:],
                    xa[:, base + 4 + s1:base + 5 + s1],
                )
            ot = o1pool.tile([P, R], BF16, tag="o1")
            w = R // n_chunks
            for c in range(n_chunks):
                for s in range(c * N_SETS // n_chunks,
                               (c + 1) * N_SETS // n_chunks):
                    nc.vector.tensor_scalar_mul(
                        ot[:, 1024 * s:1024 * (s + 1)], a1024[:], x0c(j, s)
                    )
                q.dma_start(
                    out=out_v[:, j * R + c * w:j * R + (c + 1) * w],
                    in_=ot[:, c * w:(c + 1) * w],
                )

        def emit_single(j, n_chunks, q):
            # PE-path single j: 1024-wide S, one Exp, 4 DVE finals.
            ps = ppool.tile([P, 2048], F32, tag="ps")
            for c in range(2):
                nc.tensor.matmul(
                    out=ps[:, c * 512:(c + 1) * 512],
                    lhsT=lhsT(j),
                    rhs=ohB[:, c * 512:(c + 1) * 512],
                    start=True,
                    stop=True,
                )
            e1024 = epool.tile([P, 1024], BF16, tag="e1024")
            nc.scalar.activation(e1024[:], ps[:, 0:1024], EXP)
            ot = o1pool.tile([P, R], BF16, tag="o1")
            w = R // n_chunks
            for c in range(n_chunks):
                for s in range(c * N_SETS // n_chunks,
                               (c + 1) * N_SETS // n_chunks):
                    nc.vector.tensor_scalar_mul(
                        ot[:, 1024 * s:1024 * (s + 1)], e1024[:], x0c(j, s)
                    )
                q.dma_start(
                    out=out_v[:, j * R + c * w:j * R + (c + 1) * w],
                    in_=ot[:, c * w:(c + 1) * w],
                )

        def emit_pair(ja, jb):
            ps = ppool.tile([P, 2048], F32, tag="ps")
            for idx, j in enumerate((ja, jb)):
                for c in range(2):
                    col = idx * 1024 + c * 512
                    nc.tensor.matmul(
                        out=ps[:, col:col + 512],
                        lhsT=lhsT(j),
                        rhs=ohB[:, c * 512:(c + 1) * 512],
                        start=True,
                        stop=True,
                    )
            e2048 = epool.tile([P, 2048], BF16, tag="e2048")
            nc.scalar.activation(e2048[:], ps[:], EXP)
            ot = o2pool.tile([P, 2 * R], BF16, tag="o2")
            for idx, j in enumerate((ja, jb)):
                for s in range(N_SETS):
                    nc.vector.tensor_scalar_mul(
                        ot[:, idx * R + 1024 * s:idx * R + 1024 * (s + 1)],
                        e2048[:, idx * 1024:(idx + 1) * 1024],
                        x0c(j, s),
                    )
                # post each j's 1 MB as soon as its finals are done, on
                # alternating rings: keeps the stream fed, keeps per-DMA
                # engine shares small, and lets the SDMA engines
                # round-robin between two descriptor queues.
                q = nc.sync if idx == 0 else nc.scalar
                q.dma_start(
                    out=out_v[:, j * R:(j + 1) * R],
                    in_=ot[:, idx * R:(idx + 1) * R],
                )

        emit_dve_j(0, n_chunks=4, q=nc.sync)
        emit_single(1, n_chunks=4, q=nc.scalar)
        emit_single(2, n_chunks=2, q=nc.sync)
        emit_single(3, n_chunks=2, q=nc.scalar)
        for pr in B_PAIRS:
            emit_pair(*pr)

    nc.compile()
    return nc


def _get_nc():
    if "nc" not in _CACHE:
        _CACHE["nc"] = build_nc()
    return _CACHE["nc"]


def _onehot() -> np.ndarray:
    """[40, 1024] bf16: rows v1..v5 hi (20) then lo (20); col r encodes
    (s1..s5) with s5 fastest."""
    import ml_dtypes

    r = np.arange(1024)
    o20 = np.zeros((20, 1024), dtype=np.float32)
    for v in range(1, N_VARS):
        sv = (r >> (2 * (N_VARS - 1 - v))) & 3
        for s in range(N_SETS):
            o20[(v - 1) * N_SETS + s] = (sv == s).astype(np.float32)
    return np.concatenate([o20, o20], axis=0).astype(ml_dtypes.bfloat16)


def _lcin(shard: np.ndarray) -> np.ndarray:
    """[40, 1920] bf16 log-domain hi/lo for v1..v5, j-major columns
    (col (j-1)*128+p = sample p*16+j); j0 (pure-DVE path) dropped."""
    import ml_dtypes

    t = shard[1:].transpose(0, 2, 1).reshape(20, N_SHARD)  # [(v,s), n]
    L = np.log(np.maximum(t, 1e-38)).astype(np.float32)
    hi = L.astype(ml_dtypes.bfloat16)
    lo = (L - hi.astype(np.float32)).astype(ml_dtypes.bfloat16)
    full = np.concatenate([hi, lo], axis=0)  # [40, n]
    # n = p*16 + j  ->  column (j-1)*128 + p
    full = full.reshape(KK, P, J).transpose(0, 2, 1).reshape(KK, N_SHARD)
    return np.ascontiguousarray(full[:, P:])


def _xin(shard: np.ndarray) -> np.ndarray:
    """[128, 128] f32: j0's 24-col var block | X0 for j1..15 | pad."""
    x = np.zeros((P, XIN_W), dtype=np.float32)
    m = shard.reshape(N_VARS, P, J, N_SETS)  # [v, p, j, s]
    x[:, 0:24] = m[:, :, 0, :].transpose(1, 0, 2).reshape(P, 24)
    x[:, 24:84] = m[0, :, 1:, :].reshape(P, 60)
    return np.ascontiguousarray(x)


def kernel(memberships):
    global LAST_RESULTS
    m = np.ascontiguousarray(np.asarray(memberships, dtype=np.float32))
    assert m.shape == (N_VARS, N_FULL, N_SETS), m.shape
    nc = _get_nc()
    oh = _onehot()
    shards = np.split(m, N_CORES, axis=1)
    in_maps = [
        {"xin": _xin(s), "lcin": _lcin(s), "ohin": oh} for s in shards
    ]
    res = run_bass_kernel_spmd(nc, in_maps, core_ids=list(range(N_CORES)))
    LAST_RESULTS = res
    return np.concatenate(
        [res.results[i]["out"] for i in range(N_CORES)], axis=0
    ).astype(np.float32)
